# revision 1
# baseline (speedup 1.0000x reference)
"""Trainium2 Bass kernel for Gaussian KDE evaluation.

reference math:
    val[m] = (1/N) * sum_n exp(t1 - 0.5*d2(m,n)/bw^2)
    d2(m,n) = |e_m|^2 + |b_n|^2 - 2<e_m, b_n>
    t1 = -0.5*D*log(2*pi) - log_bw,  bw^2 = exp(2*log_bw)

Strategy (8 NeuronCores, x_eval row-sharded, x_base/log_bw replicated):
  Per core, one K=17 matmul per output tile produces |b|^2 - 2<e,b> in PSUM
  (stationary lhsT rows 0..15 = -2*eval^T, row 16 = ones; moving rhs rows
  0..15 = base^T, row 16 = |b|^2).  A single ScalarE ACTIVATE computes
  exp(scale*psum + bias) in place, with scale = -0.5/bw^2 and per-partition
  bias = t1 - ln(N) + scale*|e_m|^2, and its accum_out writes the row-sum.
  log_bw is broadcast on-device via a K=1 matmul; |b_n|^2 is moved from a
  per-partition column layout to a single-partition row via a DRAM bounce.
"""

import numpy as np

M, N, D = 8192, 16384, 16
NCORES = 8
MS = M // NCORES          # eval rows per core
RT = MS // 128            # row tiles per core (128 evals each)
CH = 1536                 # main column-chunk size (3 PSUM banks)
LOG_2PI = float(np.log(2.0 * np.pi))

_CACHE = {}


def _chunks():
    out = []
    c0 = 0
    while c0 < N:
        csz = min(CH, N - c0)
        out.append((c0, csz))
        c0 += csz
    return out


def _build_nc(reps=1, loop_iters=None, skip_act=False, skip_mm=False,
              skip_tp=False, max_chunks=None):
    from concourse import bacc, mybir, masks, tile

    f32 = mybir.dt.float32
    nc = bacc.Bacc("TRN2", target_bir_lowering=False, debug=False,
                   num_devices=NCORES)

    x_eval = nc.dram_tensor("x_eval", [MS, D], f32, kind="ExternalInput")
    x_base = nc.dram_tensor("x_base", [N, D], f32, kind="ExternalInput")
    log_bw = nc.dram_tensor("log_bw", [1, 1], f32, kind="ExternalInput")
    out = nc.dram_tensor("out", [128, RT], f32, kind="ExternalOutput")
    sqb_bounce = nc.dram_tensor("sqb_bounce", [1, N], f32)

    chunks = _chunks()
    NCH = len(chunks)
    NBT = N // 128            # number of 128-row base tiles
    Exp = mybir.ActivationFunctionType.Exp
    ADD = mybir.AluOpType.add
    MULT = mybir.AluOpType.mult
    X = mybir.AxisListType.X
    # constant part of the bias: t1 - ln(N) + log_bw-dependent part added
    # on-device; c0 covers everything except -log_bw and the |e|^2 term.
    c0 = -0.5 * D * LOG_2PI - float(np.log(N))

    with tile.TileContext(nc) as tc:
        with (
            tc.tile_pool(name="persist", bufs=1) as pp,
            tc.tile_pool(name="rhs", bufs=3) as rhsp,
            tc.tile_pool(name="mm", bufs=2, space="PSUM") as mmp,
            tc.tile_pool(name="tp", bufs=2, space="PSUM") as tpp,
        ):
          from contextlib import nullcontext
          for _rep in range(reps):
           with (tc.For_i(0, loop_iters, 1) if loop_iters else nullcontext()):
            identity = pp.tile([128, 128], f32)
            masks.make_identity(nc, identity[:])

            # ---- log_bw -> per-partition scale/bias columns -------------
            ones_row = pp.tile([1, 128], f32)
            nc.vector.memset(ones_row[:], 1.0)
            lb_sb = pp.tile([1, 1], f32)
            nc.sync.dma_start(out=lb_sb[:], in_=log_bw[:])
            ps_lb = tpp.tile([128, 512], f32, tag="tp")
            nc.tensor.matmul(ps_lb[:, 0:1], ones_row[:], lb_sb[:],
                             start=True, stop=True)
            # scale = -0.5 * exp(-2*log_bw)
            inv_bw2 = pp.tile([128, 1], f32)
            nc.scalar.activation(inv_bw2[:], ps_lb[:, 0:1], Exp, scale=-2.0)
            scale_col = pp.tile([128, 1], f32)
            nc.vector.tensor_scalar_mul(scale_col[:], inv_bw2[:], -0.5)
            # c_col = c0 - log_bw
            c_col = pp.tile([128, 1], f32)
            nc.vector.tensor_scalar(out=c_col[:], in0=ps_lb[:, 0:1],
                                    scalar1=-1.0, scalar2=c0,
                                    op0=MULT, op1=ADD)

            # ---- eval-side setup ----------------------------------------
            ev_nat = pp.tile([128, RT * D], f32)
            nc.sync.dma_start(
                out=ev_nat[:].rearrange("p (t d) -> p t d", d=D),
                in_=x_eval[:].rearrange("(p t) d -> p t d", p=128))
            ev_sq = pp.tile([128, RT * D], f32)
            nc.vector.tensor_mul(ev_sq[:], ev_nat[:], ev_nat[:])
            sq_e = pp.tile([128, RT], f32)
            nc.vector.tensor_reduce(
                out=sq_e[:], in_=ev_sq[:].rearrange("p (t d) -> p t d", d=D),
                axis=X, op=ADD)
            # bias_all[:, rt] = scale*|e|^2 + (c0 - log_bw)
            bias_all = pp.tile([128, RT], f32)
            nc.vector.tensor_scalar(out=bias_all[:], in0=sq_e[:],
                                    scalar1=scale_col[:, 0:1],
                                    scalar2=c_col[:, 0:1],
                                    op0=MULT, op1=ADD)

            # evT rows 0..15 = -2 * eval^T, row 16 = ones
            evT = pp.tile([17, MS], f32)
            nc.vector.memset(evT[:], 1.0)  # row 16 stays 1.0; rows 0..15 overwritten
            for rt in range(RT):
                ps_t = tpp.tile([16, 512], f32, tag="tp")
                nc.tensor.transpose(ps_t[:, 0:128],
                                    ev_nat[:, rt * D:(rt + 1) * D],
                                    identity[:])
                nc.vector.tensor_scalar_mul(
                    evT[0:16, rt * 128:(rt + 1) * 128], ps_t[:, 0:128], -2.0)

            # ---- base load + |b|^2 row (DRAM bounce) --------------------
            bs_nat = pp.tile([128, NBT * D], f32)
            nc.sync.dma_start(
                out=bs_nat[:].rearrange("p (t d) -> p t d", d=D),
                in_=x_base[:].rearrange("(p t) d -> p t d", p=128))
            bs_sq = pp.tile([128, NBT * D], f32)
            nc.vector.tensor_mul(bs_sq[:], bs_nat[:], bs_nat[:])
            sq_b = pp.tile([128, NBT], f32)
            nc.vector.tensor_reduce(
                out=sq_b[:], in_=bs_sq[:].rearrange("p (t d) -> p t d", d=D),
                axis=X, op=ADD)
            ps_sqb = tpp.tile([128, 512], f32, tag="tp")
            nc.tensor.transpose(ps_sqb[:, 0:128], sq_b[:], identity[:])
            sq_bT = pp.tile([128, 128], f32)
            nc.vector.tensor_copy(sq_bT[:], ps_sqb[:, 0:128])
            nc.sync.dma_start(
                out=sqb_bounce[:].rearrange("o (t p) -> (o t) p", p=128),
                in_=sq_bT[:])

            # ---- main loop ----------------------------------------------
            sums = pp.tile([128, RT * NCH], f32)
            if skip_act or (max_chunks is not None and max_chunks < NCH):
                nc.vector.memset(sums[:], 0.0)
            for ci, (cs, csz) in enumerate(chunks[:max_chunks]):
                rhs = rhsp.tile([17, CH], f32, tag="rhs")
                nt = csz // 128
                for g in range((nt + 3) // 4):
                    ps_t = tpp.tile([16, 512], f32, tag="tp")
                    for j in range(min(4, nt - 4 * g)):
                        t = cs // 128 + 4 * g + j
                        if not skip_tp:
                            nc.tensor.transpose(
                                ps_t[:, j * 128:(j + 1) * 128],
                                bs_nat[:, t * D:(t + 1) * D], identity[:])
                    w = min(512, (nt - 4 * g) * 128)
                    nc.vector.tensor_copy(
                        rhs[0:16, g * 512:g * 512 + w], ps_t[:, 0:w])
                nc.sync.dma_start(out=rhs[16:17, 0:csz],
                                  in_=sqb_bounce[0:1, cs:cs + csz])
                for rt in range(RT):
                    ps = mmp.tile([128, CH], f32, tag="mm")
                    if not skip_mm:
                        for j in range(csz // 512):
                            nc.tensor.matmul(
                                ps[:, j * 512:(j + 1) * 512],
                                evT[0:17, rt * 128:(rt + 1) * 128],
                                rhs[0:17, j * 512:(j + 1) * 512],
                                start=True, stop=True)
                    if not skip_act:
                        nc.scalar.activation(
                            ps[:, 0:csz], ps[:, 0:csz], Exp,
                            bias=bias_all[:, rt:rt + 1],
                            scale=scale_col[:, 0:1],
                            accum_out=sums[:, rt * NCH + ci:rt * NCH + ci + 1])

            # ---- finalize -----------------------------------------------
            val = pp.tile([128, RT], f32)
            for rt in range(RT):
                nc.vector.tensor_reduce(
                    out=val[:, rt:rt + 1],
                    in_=sums[:, rt * NCH:(rt + 1) * NCH], axis=X, op=ADD)
            nc.sync.dma_start(out=out[:], in_=val[:])

    nc.compile()
    return nc


def kernel(x_eval, x_base, log_bw):
    from concourse.bass_utils import run_bass_kernel_spmd

    if "nc" not in _CACHE:
        _CACHE["nc"] = _build_nc()
    nc = _CACHE["nc"]

    x_eval = np.ascontiguousarray(x_eval, dtype=np.float32)
    x_base = np.ascontiguousarray(x_base, dtype=np.float32)
    lb = np.asarray(log_bw, dtype=np.float32).reshape(1, 1)
    in_maps = [
        {
            "x_eval": x_eval[i * MS:(i + 1) * MS],
            "x_base": x_base,
            "log_bw": lb,
        }
        for i in range(NCORES)
    ]
    res = run_bass_kernel_spmd(nc, in_maps, list(range(NCORES)))
    # out[p, rt] holds eval point p*RT + rt of the shard -> row-major flatten
    shards = [r["out"].reshape(-1) for r in res.results]
    return np.concatenate(shards).astype(np.float32)



# revision 2
# speedup vs baseline: 2.9942x; 2.9942x over previous
"""Trainium2 Bass kernel for Gaussian KDE evaluation.

reference math:
    val[m] = (1/N) * sum_n exp(t1 - 0.5*d2(m,n)/bw^2)
    d2(m,n) = |e_m|^2 + |b_n|^2 - 2<e_m, b_n>
    t1 = -0.5*D*log(2*pi) - log_bw,  bw^2 = exp(2*log_bw)

Strategy (8 NeuronCores, x_eval row-sharded, x_base/log_bw replicated):
  All operands are pre-scaled on host by s = sqrt(0.5)/bw so the PSUM
  matmul result is directly the (negated, biased) exponent argument:
      tau(m,n) = bias_m - PSUM(m,n),
      PSUM = |b'|^2 - 2<e',b'>,  bias_m = t1 - ln(N) - |e'_m|^2.
  The cross term runs at full PE rate (1 col/cycle) in bf16 using a
  3-term hi/lo split (eh*bh + el*bh + eh*bl); |b'|^2 rides as three
  bf16 rows against ones in lhsT.  K = 3*16 + 3 = 51.
  ScalarE does exp via ACTIVATE(scale=-1, bias) over 2048-col PSUM
  blocks (4 banks, ping-pong with the matmuls) with accum_out
  producing the row sums; a single DVE tensor_reduce finishes.
"""

import numpy as np
import ml_dtypes

M, N, D = 8192, 16384, 16
NCORES = 8
MS = M // NCORES          # eval rows per core (1024)
RT = MS // 128            # row tiles per core (8)
K = 3 * D + 3             # 51 contraction rows
CH = 2048                 # columns per exp/accum block (4 PSUM banks)
NCH = N // CH             # 8 blocks
LOG_2PI = float(np.log(2.0 * np.pi))
BF16 = ml_dtypes.bfloat16

_CACHE = {}


def _build_nc():
    from concourse import bacc, mybir, tile

    f32 = mybir.dt.float32
    bf16 = mybir.dt.bfloat16
    nc = bacc.Bacc("TRN2", target_bir_lowering=False, debug=False,
                   num_devices=NCORES)

    lhsT = nc.dram_tensor("lhsT", [K, MS], bf16, kind="ExternalInput")
    rhs = nc.dram_tensor("rhs", [K, N], bf16, kind="ExternalInput")
    bias = nc.dram_tensor("bias", [128, RT], f32, kind="ExternalInput")
    out = nc.dram_tensor("out", [128, RT], f32, kind="ExternalOutput")

    Exp = mybir.ActivationFunctionType.Exp
    ADD = mybir.AluOpType.add
    X = mybir.AxisListType.X

    with tile.TileContext(nc) as tc:
        with (
            tc.tile_pool(name="persist", bufs=1) as pp,
            tc.tile_pool(name="mm", bufs=2, space="PSUM") as mmp,
        ):
            # Warm the exp table while DMAs are in flight.
            dummy = pp.tile([1, 1], f32)
            nc.vector.memset(dummy[:], 0.0)
            nc.scalar.activation(dummy[:], dummy[:], Exp)

            lhsT_sb = pp.tile([K, MS], bf16)
            nc.sync.dma_start(out=lhsT_sb[:], in_=lhsT[:])
            bias_sb = pp.tile([128, RT], f32)
            nc.sync.dma_start(out=bias_sb[:], in_=bias[:])
            rhs_sb = pp.tile([K, N], bf16)
            for c in range(NCH):
                nc.sync.dma_start(out=rhs_sb[:, c * CH:(c + 1) * CH],
                                  in_=rhs[:, c * CH:(c + 1) * CH])

            sums = pp.tile([128, RT * NCH], f32)
            for rt in range(RT):
                for c in range(NCH):
                    ps = mmp.tile([128, CH], f32, tag="mm")
                    for j in range(CH // 512):
                        nc.tensor.matmul(
                            ps[:, j * 512:(j + 1) * 512],
                            lhsT_sb[:, rt * 128:(rt + 1) * 128],
                            rhs_sb[:, c * CH + j * 512:c * CH + (j + 1) * 512],
                            start=True, stop=True)
                    nc.scalar.activation(
                        ps[:, 0:CH], ps[:, 0:CH], Exp,
                        bias=bias_sb[:, rt:rt + 1], scale=-1.0,
                        accum_out=sums[:, rt * NCH + c:rt * NCH + c + 1])

            val = pp.tile([128, RT], f32)
            nc.vector.tensor_reduce(
                out=val[:],
                in_=sums[:].rearrange("p (r c) -> p r c", c=NCH),
                axis=X, op=ADD)
            nc.sync.dma_start(out=out[:], in_=val[:])

    nc.compile()
    return nc


def _prepare_in_maps(x_eval, x_base, log_bw):
    """Host-side operand packing (numpy): pre-scale, bf16 hi/lo split."""
    x_eval = np.ascontiguousarray(x_eval, dtype=np.float32)
    x_base = np.ascontiguousarray(x_base, dtype=np.float32)
    lb = float(np.asarray(log_bw, dtype=np.float32).reshape(-1)[0])

    s = np.sqrt(0.5 * np.exp(-2.0 * lb))
    t1 = -0.5 * D * LOG_2PI - lb

    b = (x_base.astype(np.float64) * s).astype(np.float32)
    bh = b.astype(BF16)
    bl = (b - bh.astype(np.float32)).astype(BF16)
    sqb = (b.astype(np.float64) ** 2).sum(1)
    s0 = sqb.astype(BF16)
    r = sqb - s0.astype(np.float64)
    s1 = r.astype(BF16)
    s2 = (r - s1.astype(np.float64)).astype(BF16)
    rhs = np.empty((K, N), dtype=BF16)
    rhs[0:D] = bh.T
    rhs[D:2 * D] = bh.T
    rhs[2 * D:3 * D] = bl.T
    rhs[3 * D] = s0
    rhs[3 * D + 1] = s1
    rhs[3 * D + 2] = s2

    e = (x_eval.astype(np.float64) * s).astype(np.float32)
    eh = e.astype(BF16)
    el = (e - eh.astype(np.float32)).astype(BF16)
    sqe = (e.astype(np.float64) ** 2).sum(1)
    bias_full = (t1 - np.log(N) - sqe).astype(np.float32)

    in_maps = []
    for i in range(NCORES):
        sl = slice(i * MS, (i + 1) * MS)
        lhsT = np.empty((K, MS), dtype=BF16)
        lhsT[0:D] = (-2.0 * eh[sl].astype(np.float32)).astype(BF16).T
        lhsT[D:2 * D] = (-2.0 * el[sl].astype(np.float32)).astype(BF16).T
        lhsT[2 * D:3 * D] = lhsT[0:D]
        lhsT[3 * D:] = BF16(1.0)
        # shard row r = rt*128 + p  ->  bias[p, rt]
        bias = np.ascontiguousarray(
            bias_full[sl].reshape(RT, 128).T)
        in_maps.append({"lhsT": lhsT, "rhs": rhs, "bias": bias})
    return in_maps


def _unshard(results):
    # out[p, rt] = val of shard row rt*128 + p
    shards = [np.asarray(r["out"]).T.reshape(-1) for r in results]
    return np.concatenate(shards).astype(np.float32)


def kernel(x_eval, x_base, log_bw):
    from concourse.bass_utils import run_bass_kernel_spmd

    if "nc" not in _CACHE:
        _CACHE["nc"] = _build_nc()
    nc = _CACHE["nc"]

    in_maps = _prepare_in_maps(x_eval, x_base, log_bw)
    res = run_bass_kernel_spmd(nc, in_maps, list(range(NCORES)))
    return _unshard(res.results)


# revision 3
# speedup vs baseline: 3.1240x; 1.0434x over previous
"""Trainium2 Bass kernel for Gaussian KDE evaluation.

reference math:
    val[m] = (1/N) * sum_n exp(t1 - 0.5*d2(m,n)/bw^2)
    d2(m,n) = |e_m|^2 + |b_n|^2 - 2<e_m, b_n>
    t1 = -0.5*D*log(2*pi) - log_bw,  bw^2 = exp(2*log_bw)

Strategy (8 NeuronCores, x_eval row-sharded, x_base/log_bw replicated):
  All operands are pre-scaled on host by s = sqrt(0.5)/bw so a single
  K=54 bf16 matmul produces the full negated exponent argument in PSUM:
      PSUM(m,n) = |b'|^2 - 2<e',b'> - bias_m,
      bias_m = t1 - ln(N) - |e'_m|^2,   tau = -PSUM.
  The cross term runs at full PE rate (1 col/cycle) in bf16 using a
  3-term hi/lo split (eh*bh + el*bh + eh*bl); |b'|^2 rides as three
  bf16 rows against ones in lhsT; -bias_m rides as three bf16 rows of
  lhsT against ones in rhs.  ScalarE does exp via ACTIVATE(scale=-1)
  over 2048-col PSUM blocks (4 banks, ping-ponged against the matmuls)
  with accum_out producing the row sums; one DVE tensor_reduce and a
  32B DMA finish.  The first two blocks are 512/1536 cols so compute
  starts as soon as the first small DMA lands.
"""

import numpy as np
import ml_dtypes

M, N, D = 8192, 16384, 16
NCORES = 8
MS = M // NCORES          # eval rows per core (1024)
RT = MS // 128            # row tiles per core (8)
K = 3 * D + 6             # 54: 3x16 cross + 3 |b'|^2 rows + 3 bias rows
CH = 2048                 # columns per exp/accum block (4 PSUM banks)
NCH = N // CH             # 8 blocks per row tile
SUMW = NCH + 1            # sums columns per row tile (9; rt0 uses all)
LOG_2PI = float(np.log(2.0 * np.pi))
BF16 = ml_dtypes.bfloat16

_CACHE = {}


def _blocks(rt):
    if rt == 0:
        return [(0, 512), (512, 1536)] + [(c, CH) for c in range(CH, N, CH)]
    return [(c, CH) for c in range(0, N, CH)]


_DMA_PIECES = [(0, 512), (512, 1536), (2048, 4096), (6144, 4096),
               (10240, 4096), (14336, 2048)]


def _build_nc():
    from concourse import bacc, mybir, tile

    f32 = mybir.dt.float32
    bf16 = mybir.dt.bfloat16
    nc = bacc.Bacc("TRN2", target_bir_lowering=False, debug=False,
                   num_devices=NCORES)

    lhsT = nc.dram_tensor("lhsT", [K, MS], bf16, kind="ExternalInput")
    rhs = nc.dram_tensor("rhs", [K, N], bf16, kind="ExternalInput")
    out = nc.dram_tensor("out", [128, RT], f32, kind="ExternalOutput")

    Exp = mybir.ActivationFunctionType.Exp
    ADD = mybir.AluOpType.add
    X = mybir.AxisListType.X

    with tile.TileContext(nc) as tc:
        with (
            tc.tile_pool(name="persist", bufs=1) as pp,
            tc.tile_pool(name="mm", bufs=2, space="PSUM") as mmp,
        ):
            # Warm the exp table while DMAs are in flight.
            dummy = pp.tile([1, 1], f32)
            nc.vector.memset(dummy[:], 0.0)
            nc.scalar.activation(dummy[:], dummy[:], Exp)

            lhsT_sb = pp.tile([K, MS], bf16)
            nc.sync.dma_start(out=lhsT_sb[:], in_=lhsT[:])
            rhs_sb = pp.tile([K, N], bf16)
            for c0, w in _DMA_PIECES:
                nc.sync.dma_start(out=rhs_sb[:, c0:c0 + w],
                                  in_=rhs[:, c0:c0 + w])

            sums = pp.tile([128, RT * SUMW], f32)
            nc.vector.memset(sums[:], 0.0)

            for rt in range(RT):
                for bi, (c0, w) in enumerate(_blocks(rt)):
                    ps = mmp.tile([128, CH], f32, tag="mm")
                    for j in range(w // 512):
                        nc.tensor.matmul(
                            ps[:, j * 512:(j + 1) * 512],
                            lhsT_sb[:, rt * 128:(rt + 1) * 128],
                            rhs_sb[:, c0 + j * 512:c0 + (j + 1) * 512],
                            start=True, stop=True)
                    sc = rt * SUMW + bi
                    nc.scalar.activation(
                        ps[:, 0:w], ps[:, 0:w], Exp, scale=-1.0,
                        accum_out=sums[:, sc:sc + 1])

            val = pp.tile([128, RT], f32)
            nc.vector.tensor_reduce(
                out=val[:],
                in_=sums[:].rearrange("p (r c) -> p r c", c=SUMW),
                axis=X, op=ADD)
            nc.sync.dma_start(out=out[:], in_=val[:])

    nc.compile()
    return nc


def _split3(v):
    """Split fp64 array into three bf16 parts summing to ~fp32 accuracy."""
    p0 = v.astype(BF16)
    r = v - p0.astype(np.float64)
    p1 = r.astype(BF16)
    p2 = (r - p1.astype(np.float64)).astype(BF16)
    return p0, p1, p2


def _prepare_in_maps(x_eval, x_base, log_bw):
    """Host-side operand packing (numpy): pre-scale, bf16 hi/lo split."""
    x_eval = np.ascontiguousarray(x_eval, dtype=np.float32)
    x_base = np.ascontiguousarray(x_base, dtype=np.float32)
    lb = float(np.asarray(log_bw, dtype=np.float32).reshape(-1)[0])

    s = np.sqrt(0.5 * np.exp(-2.0 * lb))
    t1 = -0.5 * D * LOG_2PI - lb

    b = (x_base.astype(np.float64) * s).astype(np.float32)
    bh = b.astype(BF16)
    bl = (b - bh.astype(np.float32)).astype(BF16)
    s0, s1, s2 = _split3((b.astype(np.float64) ** 2).sum(1))
    rhs = np.empty((K, N), dtype=BF16)
    rhs[0:D] = bh.T
    rhs[D:2 * D] = bh.T
    rhs[2 * D:3 * D] = bl.T
    rhs[3 * D] = s0
    rhs[3 * D + 1] = s1
    rhs[3 * D + 2] = s2
    rhs[3 * D + 3:] = BF16(1.0)

    e = (x_eval.astype(np.float64) * s).astype(np.float32)
    eh = e.astype(BF16)
    el = (e - eh.astype(np.float32)).astype(BF16)
    sqe = (e.astype(np.float64) ** 2).sum(1)
    # PSUM carries -bias so tau = -PSUM; v = -bias = |e'|^2 + ln(N) - t1
    v0, v1, v2 = _split3(sqe + np.log(N) - t1)

    in_maps = []
    for i in range(NCORES):
        sl = slice(i * MS, (i + 1) * MS)
        lhsT = np.empty((K, MS), dtype=BF16)
        lhsT[0:D] = (-2.0 * eh[sl].astype(np.float32)).astype(BF16).T
        lhsT[D:2 * D] = (-2.0 * el[sl].astype(np.float32)).astype(BF16).T
        lhsT[2 * D:3 * D] = lhsT[0:D]
        lhsT[3 * D:3 * D + 3] = BF16(1.0)
        lhsT[3 * D + 3] = v0[sl]
        lhsT[3 * D + 4] = v1[sl]
        lhsT[3 * D + 5] = v2[sl]
        in_maps.append({"lhsT": lhsT, "rhs": rhs})
    return in_maps


def _unshard(results):
    # out[p, rt] = val of shard row rt*128 + p
    shards = [np.asarray(r["out"]).T.reshape(-1) for r in results]
    return np.concatenate(shards).astype(np.float32)


def kernel(x_eval, x_base, log_bw):
    from concourse.bass_utils import run_bass_kernel_spmd

    if "nc" not in _CACHE:
        _CACHE["nc"] = _build_nc()
    nc = _CACHE["nc"]

    in_maps = _prepare_in_maps(x_eval, x_base, log_bw)
    res = run_bass_kernel_spmd(nc, in_maps, list(range(NCORES)))
    return _unshard(res.results)
